# revision 71
# baseline (speedup 1.0000x reference)
"""GRU-residual trajectory kernel for Trainium2 (8 NeuronCores, data-parallel).

Reference semantics (PyTorch GRUCell math):
    h' = (1-u) * n + u * h
    r  = sigmoid(W_ih_r z + b_ih_r + W_hh_r h + b_hh_r)
    u  = sigmoid(W_ih_u z + b_ih_u + W_hh_u h + b_hh_u)
    n  = tanh(W_ih_n z + b_ih_n + r * (W_hh_n h + b_hh_n))
    z' = z + dt * (W_head h' + b_head)
repeated `steps` times; output traj = [z0, z1, ..., z_steps] per batch row.

Device mapping (per core, batch shard Bc=2048, feature-major layout):
  XR [68, Bc] f32r: rows 0-63 = h state (kept ROUNDED, updated in place by
  GPSIMD — the BIR verifier accepts GPSIMD tensor ops producing an f32r
  tile), rows 64-66 = f32r copy of z (each chunk's slice refreshed every
  8th step, staggered across chunks so no step carries a whole-width z
  barrier), row 67 = ones. ZX [3, Bc] fp32 ping-pong pair = exact z
  master (the pong removes the WAR between the output DMA of step k and
  the z accumulate of step k+1). All three matmuls read XR at 1 cyc/col
  (vs 4 for fp32 — the PE-bound baseline bottleneck).
  Per step, per column chunk of 512, software-pipelined emission (engine
  queues are in-order, so stage-sorted emission = the schedule):
    G1  = W1r.T  @ XR   -> [u'-preact ; r-preact]      (PE f32r, PSUM)
    HI  = W23r.T @ XR   -> [i_n ; h_n]                 (PE f32r, PSUM)
    dz matmuls OF THE PREVIOUS STEP (same XR state)    (PE f32r)
    S   = sigmoid(G1)                                  (ACT)
    T1  = S[r] * HI[h_n] -> fp16                       (DVE)
    HI[i_n] += I @ T1                                  (PE fp16 accumulate)
    n   = tanh(HI[i_n])                                (ACT)
    T3  = n - h ; T3 *= S[u'] ; h += T3 (rounds f32r)  (GPSIMD, in-order)
  two steps deferred:  ZX' = ZX + dz (exact fp32, DVE)
                       -> one [3,2048] DMA out per step (SP HWDGE)
  Gates see z 2-9 steps stale; both the staleness and every rounding site
  (f32r state/weights, fp16 r*h_n) enter the trajectory only through
  dt-scaled integration. HW-measured rel err 4.2e-3 (gate: 2e-2).

Performance ledger (CoreSim v1 cost model; HW where noted):
  19.4ms sim / 21.96ms HW   fp32-gates baseline (PE-bound, 4 cyc/col)
  18.8ms  f32r 1-pass gates via rounded-copy reuse (DVE/ACT walls emerge)
  15.5ms  f32r h-STATE (no per-step cast op), deferred z-path, zx ping-pong
  13.9ms  stage-sorted emission, per-chunk pz banks, zcast every 4 steps
  12.7ms  unroll 64 (For_i boundary drain amortized)   [HW rel err 2.7e-3]
  12.0ms  spool/tpool bufs 6, zcast every 8 steps      [HW rel err 4.2e-3]
  11.7ms  staggered per-chunk z refresh (no z barrier)  [HW rel err 4.2e-3]
Dead ends proven: PSUM z at partition offsets 32c (HW: matmul dst must
start at partition 0); paired sigmoid/tanh PSUM tiles at bufs=1 (pipeline
serialization beats the ACT-init savings); single 4-bank pz tile
(z-chain head-of-line blocks the PE queue — refuted twice, also in the
tight final pipeline); bitcast fp32->f32r matmul inputs (BIR verifier
demands a rounding producer); z refresh on GPSIMD (Pool already at 88%).
Residual walls/step: DVE 5.26us (t1-mul + z flush, both PSUM-bound so
DVE-only), ACT 4.9us (sigmoid+tanh), Pool 5.1us (h-update) at ~92%
pipeline efficiency — a balanced three-engine plateau.
"""

import sys

for p in ("/opt/trn_rl_repo",):
    if p not in sys.path:
        sys.path.insert(0, p)

import numpy as np

import concourse.bacc as bacc
import concourse.bass as bass
import concourse.mybir as mybir
from concourse.tile import TileContext
from concourse.bass_utils import run_bass_kernel_spmd

N_CORES = 8
B_FULL = 16384
BC = B_FULL // N_CORES  # 2048 per core
D = 3
H = 64
K = H + D + 1  # 68 state rows: h (0:64), z (64:67), ones (67)
STEPS = 2048
CHUNK = 512
N_CHUNKS = BC // CHUNK
UNROLL = 64

F32 = mybir.dt.float32
F32R = mybir.dt.float32r
F16 = mybir.dt.float16
SIG = mybir.ActivationFunctionType.Sigmoid
TANH = mybir.ActivationFunctionType.Tanh
COPY = mybir.ActivationFunctionType.Copy

_NC_CACHE = {}


def _build(steps: int):
    if steps in _NC_CACHE:
        return _NC_CACHE[steps]
    nc = bacc.Bacc(None, target_bir_lowering=False)

    xr0 = nc.dram_tensor("xr0", [K, BC], F32R, kind="ExternalInput")
    z0d = nc.dram_tensor("z0d", [D, BC], F32, kind="ExternalInput")
    w1 = nc.dram_tensor("w1", [K, 2 * H], F32R, kind="ExternalInput")
    w23 = nc.dram_tensor("w23", [K, 2 * H], F32R, kind="ExternalInput")
    w5 = nc.dram_tensor("w5", [K, D], F32R, kind="ExternalInput")
    eye = nc.dram_tensor("eye16", [H, H], F16, kind="ExternalInput")
    zs = nc.dram_tensor("zs", [steps * D, BC], F32, kind="ExternalOutput")

    with TileContext(nc) as tc:
        with (
            tc.tile_pool(name="state", bufs=1) as state_pool,
            tc.tile_pool(name="wpool", bufs=1) as wpool,
            tc.tile_pool(name="spool", bufs=8) as spool,
            tc.tile_pool(name="tpool", bufs=8) as tpool,
            tc.tile_pool(name="pg1", bufs=2, space="PSUM") as pg1,
            tc.tile_pool(name="phi", bufs=2, space="PSUM") as phi,
            tc.tile_pool(name="pz", bufs=1, space="PSUM") as pz,
        ):
            xr = state_pool.tile([K, BC], F32R, tag="xr")
            # ping-pong z master: flush of step k writes zx[k%2] reading
            # zx[1-k%2], so the output DMA of step k never WAR-blocks the
            # flush of step k+1
            zxA = state_pool.tile([D, BC], F32, tag="zxA")
            zxB = state_pool.tile([D, BC], F32, tag="zxB")
            zx2 = [zxA, zxB]
            w1_t = wpool.tile([K, 2 * H], F32R, tag="w1")
            w23_t = wpool.tile([K, 2 * H], F32R, tag="w23")
            w5_t = wpool.tile([K, D], F32R, tag="w5")
            eye_t = wpool.tile([H, H], F16, tag="eye")

            nc.sync.dma_start(w1_t[:], w1[:])
            nc.sync.dma_start(w23_t[:], w23[:])
            nc.sync.dma_start(w5_t[:], w5[:])
            nc.sync.dma_start(eye_t[:], eye[:])
            nc.sync.dma_start(xr[:], xr0[:])  # h=0 | z0 | ones
            nc.sync.dma_start(zxA[:], z0d[:])  # first body's z-cast source
            nc.sync.dma_start(zxB[:], z0d[:])  # parity: flush(0) reads B

            # Pre-load the ACT spline table set covering sigmoid+tanh+copy
            # so the fixpoint pass doesn't re-load it every loop iteration.
            try:
                from concourse.hw_specs import get_activation_tables

                tabs = list(get_activation_tables(nc.m.arch).items())
                need = {SIG, TANH, COPY}
                set_id = next(
                    i for i, (_, fns) in enumerate(tabs) if need <= fns
                )
            except Exception:
                set_id = 2  # sigmoid_and_others
            nc.scalar.add_instruction(
                mybir.InstLoadActFuncSet(
                    name=nc.get_next_instruction_name(),
                    ins=[],
                    outs=[],
                    act_func_set_id=set_id,
                )
            )

            xrf = xr[:].bitcast(F32)  # fp32 view for non-matmul engines

            def emit_dz(ztiles):
                """dz matmuls for the step whose h-update just completed.
                Reads the same xr state as the next step's gate matmuls.
                Each chunk gets its own small PSUM bank (matmul dst must
                start at partition 0 on TRN2)."""
                for c in range(N_CHUNKS):
                    cs = slice(c * CHUNK, (c + 1) * CHUNK)
                    nc.tensor.matmul(
                        ztiles[c][:], w5_t[:], xr[:, cs],
                        start=True, stop=True,
                    )

            def z_flush(pending, uu):
                """z accumulate + output store for finished step uu (body
                index); writes zx[uu%2] from zx[1-uu%2]."""
                pt, ztiles = pending
                zo, zi = zx2[uu % 2], zx2[1 - uu % 2]
                for c in range(N_CHUNKS):
                    cs = slice(c * CHUNK, (c + 1) * CHUNK)
                    nc.vector.tensor_add(zo[:, cs], zi[:, cs], ztiles[c][:])
                nc.sync.dma_start(zs[bass.ds(pt * D, D), :], zo[:])

            def gru_step(dz_prev):
                """Gate path for one step; dz_prev = z-psum tiles of the
                previous step, whose matmuls are emitted interleaved with
                this step's gate matmuls (they read the same xr state)."""
                # head: all gate matmuls first (PE queue stays unblocked)
                g1s, hi_t = [], []
                for c in range(N_CHUNKS):
                    cs = slice(c * CHUNK, (c + 1) * CHUNK)
                    g1 = pg1.tile([2 * H, CHUNK], F32, tag="g1")
                    g1s.append(g1)
                    nc.tensor.matmul(
                        g1[:], w1_t[:], xr[:, cs], start=True, stop=True
                    )
                    hi = phi.tile([2 * H, CHUNK], F32, tag="hi")
                    hi_t.append(hi)
                    nc.tensor.matmul(
                        hi[:], w23_t[:], xr[:, cs], start=True, stop=True
                    )
                if dz_prev is not None:
                    emit_dz(dz_prev)

                # sigmoid per chunk, s = [u' ; r]
                sp = []
                for c in range(N_CHUNKS):
                    s = spool.tile([2 * H, CHUNK], F32, tag=f"s{c % 2}")
                    nc.scalar.activation(s[:], g1s[c][:], SIG)
                    sp.append(s)

                # t1 = r * h_n in fp16 (DVE), then PE accumulates onto i_n
                t1s = []
                for c in range(N_CHUNKS):
                    t1h = tpool.tile([H, CHUNK], F16, tag=f"t1h{c % 2}")
                    nc.vector.tensor_mul(
                        t1h[:], sp[c][H : 2 * H, :], hi_t[c][H : 2 * H, :]
                    )
                    t1s.append(t1h)
                for c in range(N_CHUNKS):
                    nc.tensor.matmul(
                        hi_t[c][0:H, :], eye_t[:], t1s[c][:],
                        start=False, stop=True, skip_group_check=True,
                    )

                # n = tanh(i_n + r*h_n) per chunk
                n_ts = []
                for c in range(N_CHUNKS):
                    n_t = tpool.tile([H, CHUNK], F32, tag=f"n{c % 2}")
                    nc.scalar.activation(n_t[:], hi_t[c][0:H, :], TANH)
                    n_ts.append(n_t)

                # h' = h + u' * (n - h); h lives rounded (f32r) in XR
                for c in range(N_CHUNKS):
                    cs = slice(c * CHUNK, (c + 1) * CHUNK)
                    t3 = tpool.tile([H, CHUNK], F32, tag=f"t3{c % 2}")
                    nc.gpsimd.tensor_sub(t3[:], n_ts[c][:], xrf[0:H, cs])
                    nc.gpsimd.tensor_mul(t3[:], t3[:], sp[c][0:H, :])
                    nc.gpsimd.tensor_add(
                        xr[0:H, cs], xrf[0:H, cs], t3[:]
                    )

            unroll = next(u for u in (UNROLL, 8, 4, 2, 1) if steps % u == 0)
            with tc.For_i(0, steps // unroll, staggered_reset=True) as tu:
                # software-pipelined within the unrolled body:
                #   body step uu emits: z-flush of step uu-2, gate matmuls
                #   of step uu with the dz matmuls of step uu-1 interleaved
                #   right after (both read the same xr state, so the PE
                #   queue never stalls on the h-update tail of the current
                #   step), and every 4th step the f32r z-rows refresh.
                # All z work for the body's steps completes inside the
                # body (no cross-iteration state except xr/zx themselves).
                pend = {}  # body-step -> (t, ztiles), set when dz emitted

                def alloc_ztiles():
                    return [
                        pz.tile([D, CHUNK], F32, tag=f"zp{c}", name=f"zp{c}")
                        for c in range(N_CHUNKS)
                    ]

                for uu in range(unroll):
                    t = tu * unroll + uu
                    if uu - 2 in pend:
                        z_flush(pend.pop(uu - 2), uu - 2)
                    if uu >= 1:
                        zt = alloc_ztiles()
                        pend[uu - 1] = (t - 1, zt)
                    else:
                        zt = None
                    gru_step(zt)
                    if uu % 2 == 0:
                        # refresh the f32r z rows one chunk at a time,
                        # staggered so no step carries a whole-width z
                        # barrier (a synchronized refresh stalled every
                        # engine ~1.8us at its cadence). Each chunk's z
                        # view refreshes every 8 steps; gates see z 2-9
                        # steps stale (enters only via dt-scaled
                        # integration). Reads the parity-0 master (the
                        # flush completed at the top of this even step).
                        c = (uu // 2) % N_CHUNKS
                        cz = slice(c * CHUNK, (c + 1) * CHUNK)
                        nc.scalar.activation(
                            xr[H : H + D, cz], zxA[:, cz], COPY
                        )
                # body tail: dz of the last step + remaining flushes
                zt = alloc_ztiles()
                pend[unroll - 1] = (tu * unroll + unroll - 1, zt)
                emit_dz(zt)
                for uu in (unroll - 2, unroll - 1):
                    z_flush(pend.pop(uu), uu)

    nc.finalize()
    _NC_CACHE[steps] = nc
    return nc


def _pack_weights(dt, W_ih, W_hh, b_ih, b_hh, W_head, b_head):
    """Host-side packing of the fused stationary weight matrices."""
    W_ih = np.asarray(W_ih, np.float32)
    W_hh = np.asarray(W_hh, np.float32)
    b_ih = np.asarray(b_ih, np.float32)
    b_hh = np.asarray(b_hh, np.float32)
    W_head = np.asarray(W_head, np.float32)
    b_head = np.asarray(b_head, np.float32)
    dt = np.float32(dt)

    ZR = slice(H, H + D)  # z rows 64:67
    ONE = K - 1  # ones row 67

    w1 = np.zeros((K, 2 * H), np.float32)
    # u gate, negated -> cols 0:H gives sigmoid(-a_u) = 1-u = u'
    w1[0:H, 0:H] = -W_hh[H : 2 * H].T
    w1[ZR, 0:H] = -W_ih[H : 2 * H].T
    w1[ONE, 0:H] = -(b_ih[H : 2 * H] + b_hh[H : 2 * H])
    # r gate -> cols H:2H
    w1[0:H, H : 2 * H] = W_hh[0:H].T
    w1[ZR, H : 2 * H] = W_ih[0:H].T
    w1[ONE, H : 2 * H] = b_ih[0:H] + b_hh[0:H]

    w23 = np.zeros((K, 2 * H), np.float32)
    # i_n -> cols 0:H (z + bias only)
    w23[ZR, 0:H] = W_ih[2 * H : 3 * H].T
    w23[ONE, 0:H] = b_ih[2 * H : 3 * H]
    # h_n -> cols H:2H (h + bias only)
    w23[0:H, H : 2 * H] = W_hh[2 * H : 3 * H].T
    w23[ONE, H : 2 * H] = b_hh[2 * H : 3 * H]

    # w5 computes only dz; exact z accumulates via DVE add in fp32 SBUF
    w5 = np.zeros((K, D), np.float32)
    w5[0:H, :] = dt * W_head.T
    w5[ONE, :] = dt * b_head
    return w1, w23, w5


def _core_feeds(z0c, w1, w23, w5):
    xr0 = np.zeros((K, BC), np.float32)
    xr0[H : H + D, :] = z0c.T
    xr0[K - 1, :] = 1.0
    return {
        "xr0": xr0,
        "z0d": np.ascontiguousarray(z0c.T),
        "w1": w1,
        "w23": w23,
        "w5": w5,
        "eye16": np.eye(H, dtype=np.float16),
    }


def _sim_feeds(np_inputs):
    """Single-core input feeds for CoreSim profiling (core 0's shard)."""
    z0 = np.asarray(np_inputs["z0"], np.float32)[:BC]
    w1, w23, w5 = _pack_weights(
        np_inputs["dt"],
        np_inputs["W_ih"],
        np_inputs["W_hh"],
        np_inputs["b_ih"],
        np_inputs["b_hh"],
        np_inputs["W_head"],
        np_inputs["b_head"],
    )
    return _core_feeds(z0, w1, w23, w5)


def kernel(z0, dt, steps, W_ih, W_hh, b_ih, b_hh, W_head, b_head):
    z0 = np.asarray(z0, np.float32)
    steps = int(steps)
    B, d = z0.shape
    assert (B, d) == (B_FULL, D)
    w1, w23, w5 = _pack_weights(dt, W_ih, W_hh, b_ih, b_hh, W_head, b_head)

    nc = _build(steps)
    in_maps = [
        _core_feeds(z0[c * BC : (c + 1) * BC], w1, w23, w5)
        for c in range(N_CORES)
    ]
    res = run_bass_kernel_spmd(nc, in_maps, core_ids=list(range(N_CORES)))

    outs = []
    for c in range(N_CORES):
        zs = res.results[c]["zs"].reshape(steps, D, BC)
        traj = np.empty((BC, steps + 1, D), np.float32)
        traj[:, 0, :] = z0[c * BC : (c + 1) * BC]
        traj[:, 1:, :] = zs.transpose(2, 0, 1)
        outs.append(traj)
    return np.concatenate(outs, axis=0)
